# revision 24
# baseline (speedup 1.0000x reference)
"""GAT (5-layer, dense-adjacency) Trainium2 kernel, sharded across 8 NeuronCores.

Sharding: query-node rows split 512/core. Per layer each core projects its
own row-block (Wh + d), AllGathers the fp32 projections, and computes its
row-block of attention with a fused DVE op (leaky(s+d)+addmask), ACT exp,
and a single float32r matmul per (j-tile, head) whose stationary packs
[ones | Wh] so softmax denominators ride along for free.
"""

import numpy as np

import concourse.bacc as bacc
import concourse.mybir as mybir
import concourse.tile as tile
from concourse.bass_utils import run_bass_kernel_spmd

import concourse.dve_ops as dve_ops
from concourse.dve_spec import Spec, Src0, Src1, C0, C1, C2, maxx, lower, PageIdx
from concourse.dve_spec import _has_src1 as _spec_has_src1
from concourse.dve_uop import DveOpSpec

dt = mybir.dt
AF = mybir.ActivationFunctionType
ALU = mybir.AluOpType

# ---------------------------------------------------------------- constants
N = 4096
NCORE = 8
ROWS = N // NCORE  # 512 query rows per core
P = 128
JT = N // P  # 32 j-tiles
NEG = -30000.0  # additive mask for non-edges; exp(x-30000) == 0
ALPHA = 0.1
# (fin, fout, heads, concat, elu_after)
CFG = [
    (256, 128, 8, True, True),
    (128, 64, 8, True, True),
    (64, 32, 4, True, True),
    (32, 16, 1, True, False),
    (16, 8, 1, False, False),
]

# ---------------------------------------------------------------- custom op
LEAKY_BIAS_ADDMASK = dve_ops.DveOp(
    "LEAKY_BIAS_ADDMASK",
    Spec(
        body=maxx(Src0 + C0, (Src0 + C0) * C1) + Src1,
        reference=lambda in0, in1, s0, s1, imm2: (
            np.maximum(in0 + s0, (in0 + s0) * s1) + in1
        ).astype(np.float32),
    ),
    subdim=False,
    uops_sha={},
)


def _register_custom_op(op):
    if op.name in dve_ops._SUB_OPCODE_FOR_NAME:
        return
    idx = dve_ops._CUSTOM_DVE_ROW_BASE + len(dve_ops.OPS)
    assert idx < 0x20
    dve_ops.OPS.append(op)
    dve_ops.CUSTOM_DVE_SPECS[op.name] = op.spec
    dve_ops._SUB_OPCODE_FOR_NAME[op.name] = idx
    shas = {}
    for ver in ("v3", "v4"):
        try:
            s = DveOpSpec(
                name=op.name,
                opcode=idx,
                uops=lower(op.spec, ver=ver),
                rd1_en=_spec_has_src1(op.spec),
            )
            shas[ver] = s.sha(ver)
        except Exception:
            pass
    object.__setattr__(op, "uops_sha", shas)


_register_custom_op(LEAKY_BIAS_ADDMASK)


def _leaky2_ref(in0, in1, s0, s1, imm2):
    x = in0.reshape(in0.shape[0], 2, -1).astype(np.float32)
    d0 = np.asarray(s0).reshape(-1, 1)
    dd = np.asarray(s1).reshape(-1, 1)
    t = np.stack([x[:, 0, :] + d0, x[:, 1, :] + d0 + dd], axis=1)
    l = np.maximum(t, t * imm2) + in1.reshape(t.shape).astype(np.float32)
    return l.reshape(in0.shape).astype(np.float32)


_T2 = Src0 + PageIdx(C0, C1)
LEAKY2_BIAS_ADDMASK = dve_ops.DveOp(
    "LEAKY2_BIAS_ADDMASK",
    Spec(body=maxx(_T2, _T2 * C2) + Src1, reference=_leaky2_ref),
    subdim=True,
    uops_sha={},
)
_register_custom_op(LEAKY2_BIAS_ADDMASK)


def _layer_split(h, dh):
    """AG split: group A (first hA heads + all d cols), group B (rest)."""
    hA = min(2, h)
    hB = h - hA
    CWa = hA * dh + h
    CWb = hB * dh
    return hA, hB, CWa, CWb


def _att_groups(h, hA):
    """Attention head groups: first group = AG-A heads, rest from AG-B."""
    if h == 1:
        return [[0]]
    gs = [list(range(hA))]
    rest = list(range(hA, h))
    while rest:
        take = min(4, len(rest))
        gs.append(rest[:take])
        rest = rest[take:]
    return gs


# ---------------------------------------------------------------- builder
def build_kernel():
    nc = bacc.Bacc("TRN2", target_bir_lowering=False, debug=False)

    adjT = nc.dram_tensor("adjT", [N, ROWS], dt.int8, kind="ExternalInput")
    x0T_own = nc.dram_tensor("x0T_own", [256, ROWS], dt.float32r, kind="ExternalInput")
    wext_dram = {}
    ws_dram = {}
    for li, (fin, fout, h, concat, _elu) in enumerate(CFG, start=1):
        dh = fout // h if concat else fout
        CWp = h * dh + h + (h * dh + h) % 2
        wext_dram[li] = nc.dram_tensor(
            f"wext{li}", [fin, CWp], dt.float32r, kind="ExternalInput"
        )
        ws_dram[li] = nc.dram_tensor(f"ws{li}", [fin, h], dt.float32r, kind="ExternalInput")

    pool_out = nc.dram_tensor("pool_part", [8, 1], dt.float32, kind="ExternalOutput")

    with tile.TileContext(nc) as tc:
        with (
            tc.tile_pool(name="persist", bufs=1) as persist,
            tc.tile_pool(name="dram", bufs=1, space="DRAM") as drampool,
            tc.tile_pool(name="dramsh", bufs=1, space="DRAM") as drampool_sh,
            tc.tile_pool(name="xTown", bufs=3) as xTown_pool,
            tc.tile_pool(name="layerbuf", bufs=1) as layerbuf,
            tc.tile_pool(name="mstage", bufs=6) as mstage,
            tc.tile_pool(name="work", bufs=2) as work,
            tc.tile_pool(name="sflat", bufs=1) as sflat_pool,
            tc.tile_pool(name="small", bufs=2) as small,
            tc.tile_pool(name="whps", bufs=1, space="PSUM") as whps,
            tc.tile_pool(name="sps", bufs=1, space="PSUM") as sps,
            tc.tile_pool(name="attps", bufs=4, space="PSUM") as attps,
        ):
            # ---------------- persistent tiles
            maskT = persist.tile([P, JT, ROWS], dt.bfloat16, tag="maskT")
            ones_row32 = persist.tile([1, P], dt.float32, tag="ones_row32")
            nc.vector.memset(ones_row32[:], 1.0)
            ones_row = persist.tile([1, P], dt.float32r, tag="ones_row")
            nc.vector.tensor_copy(ones_row[:], ones_row32[:])
            ones_col = persist.tile([P, JT], dt.float32, tag="ones_col")
            nc.vector.memset(ones_col[:], 1.0)
            
            wext_sb = {}
            ws_sb = {}
            for li, (fin, fout, h, concat, _elu) in enumerate(CFG, start=1):
                dh = fout // h if concat else fout
                nft = (fin + P - 1) // P
                wext_sb[li] = []
                ws_sb[li] = []
                for ft in range(nft):
                    fr = min(P, fin - ft * P)
                    wt = persist.tile([fr, h * dh + h + (h * dh + h) % 2], dt.float32r, tag=f"wext{li}_{ft}")
                    nc.sync.dma_start(wt[:], wext_dram[li][ft * P : ft * P + fr, :])
                    wext_sb[li].append(wt)
                    st = persist.tile([fr, h], dt.float32r, tag=f"ws{li}_{ft}")
                    nc.sync.dma_start(st[:], ws_dram[li][ft * P : ft * P + fr, :])
                    ws_sb[li].append(st)

            # ---------------- L1 own activations from input
            xTown_cur = []
            for ft in range(2):
                to = xTown_pool.tile([P, ROWS], dt.float32r, tag="xTown")
                nc.sync.dma_start(to[:], x0T_own[ft * P : (ft + 1) * P, :])
                xTown_cur.append(to)

            for li, (fin, fout, h, concat, elu) in enumerate(CFG, start=1):
                dh = fout // h if concat else fout
                hdh = h * dh
                CW = hdh + h  # Wh values + d column(s)
                CWp = CW + CW % 2  # fp32r matmul needs even moving dim
                nft = (fin + P - 1) // P
                is_last = li == len(CFG)
                hA, hB, CWa, CWb = _layer_split(h, dh)
                SW = dh + 1  # stationary width per head: [ones | Wh]

                # ---- (A) own-block Wh (+d) for the 4 own j-chunks (fp32)
                own_sb = work.tile([P, 4, CWp], dt.float32r, tag="own_sb")
                for k in range(4):
                    pw = whps.tile([P, CWp], dt.float32, tag="pw")
                    for ft in range(nft):
                        fr = min(P, fin - ft * P)
                        nc.tensor.matmul(
                            pw[:],
                            xTown_cur[ft][0:fr, k * P : (k + 1) * P],
                            wext_sb[li][ft][:],
                            start=(ft == 0),
                            stop=(ft == nft - 1),
                        )
                    nc.scalar.copy(own_sb[:, k, :], pw[:])

                # ---- (C) AllGather fp32 projections (A then B)
                ag_a_in = drampool.tile([4 * P, CWa], dt.float32r, tag=f"again{li}")
                ag_a_out = drampool_sh.tile(
                    [NCORE, 4 * P, CWa], dt.float32r, tag=f"agaout{li}",
                    addr_space="Shared",
                )
                nc.sync.dma_start(
                    ag_a_in.rearrange("(k p) c -> p k c", p=P), own_sb[:, :, 0:CWa]
                )
                nc.gpsimd.collective_compute(
                    "AllGather",
                    mybir.AluOpType.bypass,
                    replica_groups=[list(range(NCORE))],
                    ins=[ag_a_in.opt()],
                    outs=[ag_a_out.opt()],
                )
                if hB:
                    ag_b_in = drampool.tile([4 * P, CWb], dt.float32r, tag=f"agbin{li}")
                    ag_b_out = drampool_sh.tile(
                        [NCORE, 4 * P, CWb], dt.float32r, tag=f"agbout{li}",
                        addr_space="Shared",
                    )
                    nc.sync.dma_start(
                        ag_b_in.rearrange("(k p) c -> p k c", p=P),
                        own_sb[:, :, CWa:CW],
                    )
                    nc.gpsimd.collective_compute(
                        "AllGather",
                        mybir.AluOpType.bypass,
                        replica_groups=[list(range(NCORE))],
                        ins=[ag_b_in.opt()],
                        outs=[ag_b_out.opt()],
                    )

                if li == 1:
                    # mask build overlaps L1's AllGather: adjT rows are j
                    # (host-transposed); cast+scale on gpsimd, DVE helps.
                    for jt in range(JT):
                        stage_i = mstage.tile([P, ROWS], dt.int8, tag="stage_i")
                        nc.sync.dma_start(stage_i[:], adjT[jt * P : (jt + 1) * P, :])
                        eng = nc.gpsimd
                        eng.tensor_scalar(
                            maskT[:, jt, :], stage_i[:], -NEG, NEG, ALU.mult, ALU.add
                        )

                # ---- (B) s rows from own activations + srep broadcasts
                ps_s = sps.tile([h, ROWS], dt.float32, tag="ps_s")
                for ft in range(nft):
                    fr = min(P, fin - ft * P)
                    nc.tensor.matmul(
                        ps_s[:],
                        ws_sb[li][ft][:],
                        xTown_cur[ft][0:fr, :],
                        start=(ft == 0),
                        stop=(ft == nft - 1),
                    )
                s_rows = small.tile([h, ROWS], dt.float32r, tag="s_rows")
                nc.vector.tensor_copy(s_rows[:], ps_s[:])
                s_flat = sflat_pool.tile([1, h, ROWS], dt.float32r, tag="s_flat")
                nc.sync.dma_start(s_flat[:], s_rows[:])
                srep_all = layerbuf.tile([P, h, ROWS], dt.float32, tag="srep_all")
                for hh in range(h):
                    ps_rep = sps.tile([P, ROWS], dt.float32, tag="ps_rep")
                    nc.tensor.matmul(
                        ps_rep[:], ones_row[:],
                        s_flat[0:1, hh, :],
                        start=True, stop=True,
                    )
                    nc.scalar.copy(srep_all[:, hh, :], ps_rep[:])

                # ---- (D) unpack: direct strided DMAs into matmul layout
                # One whrow tile per attention group so group-A attention can
                # start while AG-B is still in flight (tile-granular deps).
                groups = _att_groups(h, hA)
                whrow_g = []
                for gi, gheads in enumerate(groups):
                    wg = layerbuf.tile(
                        [P, JT, len(gheads), SW], dt.float32r, tag=f"whrow{gi}"
                    )
                    for kk in range(len(gheads)):
                        nc.vector.tensor_copy(wg[:, :, kk, 0:1], ones_col[:, :])
                    whrow_g.append(wg)
                d_sb = layerbuf.tile([P, JT, h], dt.float32r, tag="d_sb")
                HR = NCORE // 2
                for half in range(2):
                    rs, js = half * HR, half * HR * 4
                    nc.sync.dma_start(
                        d_sb[:, js : js + 4 * HR, :],
                        ag_a_out[rs : rs + HR, :, hA * dh : hA * dh + h].rearrange(
                            "r (k p) h -> p (r k) h", p=P
                        ),
                    )
                d_diff = layerbuf.tile([P, JT // 2, h], dt.float32, tag="d_diff")
                nc.vector.tensor_sub(
                    d_diff[:],
                    d_sb[:, 1::2, :].bitcast(dt.float32),
                    d_sb[:, 0::2, :].bitcast(dt.float32),
                )
                for gi, gheads in enumerate(groups):
                    for kk, hh in enumerate(gheads):
                        if hh < hA:
                            src_ap = ag_a_out[:, :, hh * dh : (hh + 1) * dh]
                        else:
                            src_ap = ag_b_out[
                                :, :, (hh - hA) * dh : (hh - hA + 1) * dh
                            ]
                        for half in range(2):
                            rs, js = half * HR, half * HR * 4
                            nc.sync.dma_start(
                                whrow_g[gi][:, js : js + 4 * HR, kk, 1 : dh + 1],
                                src_ap[rs : rs + HR].rearrange(
                                    "r (k p) d -> p (r k) d", p=P
                                ),
                            )

                # ---- (F) attention per head group
                xnext = xTown_pool.tile([fout, ROWS], dt.float32r, tag="xTown")
                for gi, gs in enumerate(groups):
                    ng = len(gs)
                    att_acc = []
                    for _k in gs:
                        att_t = attps.tile([SW, ROWS], dt.float32, tag="att")
                        att_acc.append(att_t)
                    for jt2 in range(0, JT, 2):
                        l_jt = work.tile([P, ng, 2 * ROWS], dt.bfloat16, tag="l_jt")
                        for k, hh in enumerate(gs):
                            nc.vector._custom_dve(
                                LEAKY2_BIAS_ADDMASK,
                                out=l_jt[:, k, :].rearrange(
                                    "p (s n) -> p s n", s=2
                                ),
                                in0=srep_all[:, hh, :]
                                .rearrange("p (o n) -> p o n", o=1)
                                .broadcast_to([P, 2, ROWS]),
                                in1=maskT[:, jt2 : jt2 + 2, :].rearrange(
                                    "p s n -> p (s n)"
                                ),
                                s0=d_sb[:, jt2, hh : hh + 1].bitcast(dt.float32),
                                s1=d_diff[:, jt2 // 2, hh : hh + 1],
                                imm2=ALPHA,
                            )
                        p_jt = work.tile([P, ng, 2 * ROWS], dt.float32r, tag="p_jt")
                        nc.scalar.activation(p_jt[:], l_jt[:], AF.Exp)
                        for s in range(2):
                            for k, hh in enumerate(gs):
                                nc.tensor.matmul(
                                    att_acc[k][:],
                                    whrow_g[gi][:, jt2 + s, k, :],
                                    p_jt[:, k, s * ROWS : (s + 1) * ROWS],
                                    start=(jt2 == 0 and s == 0),
                                    stop=(jt2 == JT - 2 and s == 1),
                                )
                    # epilogue per head: normalize; write into xnext via DMA
                    for k, hh in enumerate(gs):
                        o_sb = small.tile([SW, ROWS], dt.float32, tag="o_sb")
                        nc.scalar.copy(o_sb[:], att_acc[k][:])
                        r_sb = small.tile([1, ROWS], dt.float32, tag="vec1")
                        nc.vector.reciprocal_approx_fast(r_sb[:], o_sb[0:1, :])
                        ps_rr = sps.tile([SW, ROWS], dt.float32, tag="ps_rep")
                        nc.tensor.matmul(
                            ps_rr[:], ones_row32[0:1, 0:SW],
                            r_sb[:],
                            start=True, stop=True,
                        )
                        onorm = small.tile([SW, ROWS], dt.float32r, tag="onorm")
                        nc.vector.tensor_mul(onorm[:], o_sb[:], ps_rr[:])
                        nc.sync.dma_start(
                            xnext[hh * dh : (hh + 1) * dh, :], onorm[1 : dh + 1, :]
                        )

                if elu:
                    # elu(x) = max(x,0) - 1 + exp(min(x,0)), batched full-width
                    mmin = small.tile([fout, ROWS], dt.float32, tag="emm")
                    nc.vector.tensor_scalar(
                        mmin[:], xnext[:], 0.0, None, ALU.min
                    )
                    emin = small.tile([fout, ROWS], dt.float32, tag="emm2")
                    nc.scalar.activation(emin[:], mmin[:], AF.Exp)
                    nc.vector.tensor_scalar(
                        mmin[:], xnext[:], 0.0, -1.0, ALU.max, ALU.add
                    )
                    nc.vector.tensor_add(xnext[:], mmin[:], emin[:])

                if is_last:
                    psum_final = small.tile([fout, 1], dt.float32, tag="vec1f")
                    nc.vector.reduce_sum(
                        psum_final[:], xnext[:], axis=mybir.AxisListType.X
                    )
                    nc.sync.dma_start(pool_out[:], psum_final[:])
                else:
                    ftiles = []
                    for ft in range((fout + P - 1) // P):
                        fr = min(P, fout - ft * P)
                        ftiles.append(xnext[ft * P : ft * P + fr, :])
                    xTown_cur = ftiles

    nc.finalize()
    return nc


_NC_CACHE = None
_last_in_maps = None


def postprocess(results, inputs):
    fc_w = np.asarray(inputs["fc_w"], dtype=np.float32)
    fc_b = np.asarray(inputs["fc_b"], dtype=np.float32)
    total = np.zeros((8,), dtype=np.float32)
    for c in range(NCORE):
        total += results[c]["pool_part"][:, 0]
    pooled = total / np.float32(N)
    return (pooled @ fc_w + fc_b).astype(np.float32)


def kernel(**inputs):
    global _NC_CACHE, _last_in_maps
    node_features = np.asarray(inputs["node_features"], dtype=np.float32)
    adj = np.asarray(inputs["adj_mat"], dtype=np.int32)
    adjT = np.ascontiguousarray(adj.T)
    x0T = np.ascontiguousarray(node_features.T)  # [256, N]

    wext = {}
    ws = {}
    for li, (fin, fout, h, concat, _elu) in enumerate(CFG, start=1):
        dh = fout // h if concat else fout
        hA, hB, CWa, CWb = _layer_split(h, dh)
        W = np.asarray(inputs[f"W{li}"], dtype=np.float32)  # [h, fin, dh]
        a_src = np.asarray(inputs[f"a_src{li}"], dtype=np.float32)  # [h, dh]
        a_dst = np.asarray(inputs[f"a_dst{li}"], dtype=np.float32)
        wcat = W.transpose(1, 0, 2).reshape(fin, h * dh)
        wd = np.einsum("hfd,hd->fh", W, a_dst).astype(np.float32)
        wsrc = np.einsum("hfd,hd->fh", W, a_src).astype(np.float32)
        wcat_full = np.concatenate([wcat[:, : hA * dh], wd, wcat[:, hA * dh :]], axis=1)
        if wcat_full.shape[1] % 2:
            wcat_full = np.concatenate(
                [wcat_full, np.zeros((fin, 1), np.float32)], axis=1
            )
        wext[li] = np.ascontiguousarray(wcat_full)
        ws[li] = np.ascontiguousarray(wsrc)

    in_maps = []
    for c in range(NCORE):
        m = {
            "adjT": np.ascontiguousarray(adjT[:, c * ROWS : (c + 1) * ROWS]).astype(np.int8),
            "x0T_own": np.ascontiguousarray(x0T[:, c * ROWS : (c + 1) * ROWS]),
        }
        for li in range(1, 6):
            m[f"wext{li}"] = wext[li]
            m[f"ws{li}"] = ws[li]
        in_maps.append(m)

    if _NC_CACHE is None:
        _NC_CACHE = build_kernel()
    nc = _NC_CACHE
    _last_in_maps = in_maps

    res = run_bass_kernel_spmd(nc, in_maps, list(range(NCORE)))
    return postprocess(res.results, inputs)


# revision 25
# speedup vs baseline: 1.0120x; 1.0120x over previous
"""GAT (5-layer, dense-adjacency) Trainium2 kernel, sharded across 8 NeuronCores.

Sharding: query-node rows split 512/core. Per layer each core projects its
own row-block (Wh + d), AllGathers the fp32 projections, and computes its
row-block of attention with a fused DVE op (leaky(s+d)+addmask), ACT exp,
and a single float32r matmul per (j-tile, head) whose stationary packs
[ones | Wh] so softmax denominators ride along for free.
"""

import numpy as np

import concourse.bacc as bacc
import concourse.mybir as mybir
import concourse.tile as tile
from concourse.bass_utils import run_bass_kernel_spmd

import concourse.dve_ops as dve_ops
from concourse.dve_spec import Spec, Src0, Src1, C0, C1, C2, maxx, lower, PageIdx
from concourse.dve_spec import _has_src1 as _spec_has_src1
from concourse.dve_uop import DveOpSpec

dt = mybir.dt
AF = mybir.ActivationFunctionType
ALU = mybir.AluOpType

# ---------------------------------------------------------------- constants
N = 4096
NCORE = 8
ROWS = N // NCORE  # 512 query rows per core
P = 128
JT = N // P  # 32 j-tiles
NEG = -30000.0  # additive mask for non-edges; exp(x-30000) == 0
ALPHA = 0.1
# (fin, fout, heads, concat, elu_after)
CFG = [
    (256, 128, 8, True, True),
    (128, 64, 8, True, True),
    (64, 32, 4, True, True),
    (32, 16, 1, True, False),
    (16, 8, 1, False, False),
]

# ---------------------------------------------------------------- custom op
LEAKY_BIAS_ADDMASK = dve_ops.DveOp(
    "LEAKY_BIAS_ADDMASK",
    Spec(
        body=maxx(Src0 + C0, (Src0 + C0) * C1) + Src1,
        reference=lambda in0, in1, s0, s1, imm2: (
            np.maximum(in0 + s0, (in0 + s0) * s1) + in1
        ).astype(np.float32),
    ),
    subdim=False,
    uops_sha={},
)


def _register_custom_op(op):
    if op.name in dve_ops._SUB_OPCODE_FOR_NAME:
        return
    idx = dve_ops._CUSTOM_DVE_ROW_BASE + len(dve_ops.OPS)
    assert idx < 0x20
    dve_ops.OPS.append(op)
    dve_ops.CUSTOM_DVE_SPECS[op.name] = op.spec
    dve_ops._SUB_OPCODE_FOR_NAME[op.name] = idx
    shas = {}
    for ver in ("v3", "v4"):
        try:
            s = DveOpSpec(
                name=op.name,
                opcode=idx,
                uops=lower(op.spec, ver=ver),
                rd1_en=_spec_has_src1(op.spec),
            )
            shas[ver] = s.sha(ver)
        except Exception:
            pass
    object.__setattr__(op, "uops_sha", shas)


_register_custom_op(LEAKY_BIAS_ADDMASK)


def _leaky2_ref(in0, in1, s0, s1, imm2):
    x = in0.reshape(in0.shape[0], 2, -1).astype(np.float32)
    d0 = np.asarray(s0).reshape(-1, 1)
    dd = np.asarray(s1).reshape(-1, 1)
    t = np.stack([x[:, 0, :] + d0, x[:, 1, :] + d0 + dd], axis=1)
    l = np.maximum(t, t * imm2) + in1.reshape(t.shape).astype(np.float32)
    return l.reshape(in0.shape).astype(np.float32)


_T2 = Src0 + PageIdx(C0, C1)
LEAKY2_BIAS_ADDMASK = dve_ops.DveOp(
    "LEAKY2_BIAS_ADDMASK",
    Spec(body=maxx(_T2, _T2 * C2) + Src1, reference=_leaky2_ref),
    subdim=True,
    uops_sha={},
)
_register_custom_op(LEAKY2_BIAS_ADDMASK)


def _layer_split(h, dh):
    """AG split: group A (first hA heads + all d cols), group B (rest)."""
    hA = min(2, h)
    hB = h - hA
    CWa = hA * dh + h
    CWb = hB * dh
    return hA, hB, CWa, CWb


def _att_groups(h, hA):
    """Attention head groups: first group = AG-A heads, rest from AG-B."""
    if h == 1:
        return [[0]]
    gs = [list(range(hA))]
    rest = list(range(hA, h))
    while rest:
        take = min(4, len(rest))
        gs.append(rest[:take])
        rest = rest[take:]
    return gs


# ---------------------------------------------------------------- builder
def build_kernel():
    nc = bacc.Bacc("TRN2", target_bir_lowering=False, debug=False)

    adjT = nc.dram_tensor("adjT", [N, ROWS], dt.int8, kind="ExternalInput")
    x0T_own = nc.dram_tensor("x0T_own", [256, ROWS], dt.float32r, kind="ExternalInput")
    wext_dram = {}
    ws_dram = {}
    for li, (fin, fout, h, concat, _elu) in enumerate(CFG, start=1):
        dh = fout // h if concat else fout
        CWp = h * dh + h + (h * dh + h) % 2
        wext_dram[li] = nc.dram_tensor(
            f"wext{li}", [fin, CWp], dt.float32r, kind="ExternalInput"
        )
        ws_dram[li] = nc.dram_tensor(f"ws{li}", [fin, h], dt.float32r, kind="ExternalInput")

    pool_out = nc.dram_tensor("pool_part", [8, 1], dt.float32, kind="ExternalOutput")

    with tile.TileContext(nc) as tc:
        with (
            tc.tile_pool(name="persist", bufs=1) as persist,
            tc.tile_pool(name="dram", bufs=1, space="DRAM") as drampool,
            tc.tile_pool(name="dramsh", bufs=1, space="DRAM") as drampool_sh,
            tc.tile_pool(name="xTown", bufs=3) as xTown_pool,
            tc.tile_pool(name="layerbuf", bufs=1) as layerbuf,
            tc.tile_pool(name="mstage", bufs=6) as mstage,
            tc.tile_pool(name="work", bufs=2) as work,
            tc.tile_pool(name="sflat", bufs=1) as sflat_pool,
            tc.tile_pool(name="small", bufs=2) as small,
            tc.tile_pool(name="whps", bufs=1, space="PSUM") as whps,
            tc.tile_pool(name="sps", bufs=1, space="PSUM") as sps,
            tc.tile_pool(name="attps", bufs=4, space="PSUM") as attps,
        ):
            # ---------------- persistent tiles
            maskT = persist.tile([P, JT, ROWS], dt.bfloat16, tag="maskT")
            ones_row32 = persist.tile([1, P], dt.float32, tag="ones_row32")
            nc.vector.memset(ones_row32[:], 1.0)
            ones_row = persist.tile([1, P], dt.float32r, tag="ones_row")
            nc.vector.tensor_copy(ones_row[:], ones_row32[:])
            ones_col = persist.tile([P, JT], dt.float32, tag="ones_col")
            nc.vector.memset(ones_col[:], 1.0)
            
            wext_sb = {}
            ws_sb = {}
            for li, (fin, fout, h, concat, _elu) in enumerate(CFG, start=1):
                dh = fout // h if concat else fout
                nft = (fin + P - 1) // P
                wext_sb[li] = []
                ws_sb[li] = []
                for ft in range(nft):
                    fr = min(P, fin - ft * P)
                    wt = persist.tile([fr, h * dh + h + (h * dh + h) % 2], dt.float32r, tag=f"wext{li}_{ft}")
                    nc.sync.dma_start(wt[:], wext_dram[li][ft * P : ft * P + fr, :])
                    wext_sb[li].append(wt)
                    st = persist.tile([fr, h], dt.float32r, tag=f"ws{li}_{ft}")
                    nc.sync.dma_start(st[:], ws_dram[li][ft * P : ft * P + fr, :])
                    ws_sb[li].append(st)

            # ---------------- L1 own activations from input
            xTown_cur = []
            for ft in range(2):
                to = xTown_pool.tile([P, ROWS], dt.float32r, tag="xTown")
                nc.sync.dma_start(to[:], x0T_own[ft * P : (ft + 1) * P, :])
                xTown_cur.append(to)

            for li, (fin, fout, h, concat, elu) in enumerate(CFG, start=1):
                dh = fout // h if concat else fout
                hdh = h * dh
                CW = hdh + h  # Wh values + d column(s)
                CWp = CW + CW % 2  # fp32r matmul needs even moving dim
                nft = (fin + P - 1) // P
                is_last = li == len(CFG)
                hA, hB, CWa, CWb = _layer_split(h, dh)
                SW = dh + 1  # stationary width per head: [ones | Wh]

                # ---- (A) own-block Wh (+d) for the 4 own j-chunks (fp32)
                own_sb = work.tile([P, 4, CWp], dt.float32r, tag="own_sb")
                for k in range(4):
                    pw = whps.tile([P, CWp], dt.float32, tag="pw")
                    for ft in range(nft):
                        fr = min(P, fin - ft * P)
                        nc.tensor.matmul(
                            pw[:],
                            xTown_cur[ft][0:fr, k * P : (k + 1) * P],
                            wext_sb[li][ft][:],
                            start=(ft == 0),
                            stop=(ft == nft - 1),
                        )
                    nc.scalar.copy(own_sb[:, k, :], pw[:])

                # ---- (C) AllGather fp32 projections (A then B)
                ag_a_in = drampool.tile([4 * P, CWa], dt.float32r, tag=f"again{li}")
                ag_a_out = drampool_sh.tile(
                    [NCORE, 4 * P, CWa], dt.float32r, tag=f"agaout{li}",
                    addr_space="Shared",
                )
                nc.sync.dma_start(
                    ag_a_in.rearrange("(k p) c -> p k c", p=P), own_sb[:, :, 0:CWa]
                )
                nc.gpsimd.collective_compute(
                    "AllGather",
                    mybir.AluOpType.bypass,
                    replica_groups=[list(range(NCORE))],
                    ins=[ag_a_in.opt()],
                    outs=[ag_a_out.opt()],
                )
                if hB:
                    ag_b_in = drampool.tile([4 * P, CWb], dt.float32r, tag=f"agbin{li}")
                    ag_b_out = drampool_sh.tile(
                        [NCORE, 4 * P, CWb], dt.float32r, tag=f"agbout{li}",
                        addr_space="Shared",
                    )
                    nc.sync.dma_start(
                        ag_b_in.rearrange("(k p) c -> p k c", p=P),
                        own_sb[:, :, CWa:CW],
                    )
                    nc.gpsimd.collective_compute(
                        "AllGather",
                        mybir.AluOpType.bypass,
                        replica_groups=[list(range(NCORE))],
                        ins=[ag_b_in.opt()],
                        outs=[ag_b_out.opt()],
                    )

                if li == 1:
                    # mask build overlaps L1's AllGather: adjT rows are j
                    # (host-transposed); cast+scale on gpsimd, DVE helps.
                    for jt in range(JT):
                        stage_i = mstage.tile([P, ROWS], dt.int8, tag="stage_i")
                        nc.sync.dma_start(stage_i[:], adjT[jt * P : (jt + 1) * P, :])
                        eng = nc.gpsimd if jt % 4 else nc.vector
                        eng.tensor_scalar(
                            maskT[:, jt, :], stage_i[:], -NEG, NEG, ALU.mult, ALU.add
                        )

                # ---- (B) s rows from own activations + srep broadcasts
                ps_s = sps.tile([h, ROWS], dt.float32, tag="ps_s")
                for ft in range(nft):
                    fr = min(P, fin - ft * P)
                    nc.tensor.matmul(
                        ps_s[:],
                        ws_sb[li][ft][:],
                        xTown_cur[ft][0:fr, :],
                        start=(ft == 0),
                        stop=(ft == nft - 1),
                    )
                s_rows = small.tile([h, ROWS], dt.float32r, tag="s_rows")
                nc.vector.tensor_copy(s_rows[:], ps_s[:])
                s_flat = sflat_pool.tile([1, h, ROWS], dt.float32r, tag="s_flat")
                nc.sync.dma_start(s_flat[:], s_rows[:])
                srep_all = layerbuf.tile([P, h, ROWS], dt.float32, tag="srep_all")
                for hh in range(h):
                    ps_rep = sps.tile([P, ROWS], dt.float32, tag="ps_rep")
                    nc.tensor.matmul(
                        ps_rep[:], ones_row[:],
                        s_flat[0:1, hh, :],
                        start=True, stop=True,
                    )
                    nc.scalar.copy(srep_all[:, hh, :], ps_rep[:])

                # ---- (D) unpack: direct strided DMAs into matmul layout
                # One whrow tile per attention group so group-A attention can
                # start while AG-B is still in flight (tile-granular deps).
                groups = _att_groups(h, hA)
                whrow_g = []
                for gi, gheads in enumerate(groups):
                    wg = layerbuf.tile(
                        [P, JT, len(gheads), SW], dt.float32r, tag=f"whrow{gi}"
                    )
                    for kk in range(len(gheads)):
                        nc.vector.tensor_copy(wg[:, :, kk, 0:1], ones_col[:, :])
                    whrow_g.append(wg)
                d_sb = layerbuf.tile([P, JT, h], dt.float32r, tag="d_sb")
                HR = NCORE // 2
                for half in range(2):
                    rs, js = half * HR, half * HR * 4
                    nc.sync.dma_start(
                        d_sb[:, js : js + 4 * HR, :],
                        ag_a_out[rs : rs + HR, :, hA * dh : hA * dh + h].rearrange(
                            "r (k p) h -> p (r k) h", p=P
                        ),
                    )
                d_diff = layerbuf.tile([P, JT // 2, h], dt.float32, tag="d_diff")
                nc.vector.tensor_sub(
                    d_diff[:],
                    d_sb[:, 1::2, :].bitcast(dt.float32),
                    d_sb[:, 0::2, :].bitcast(dt.float32),
                )
                for gi, gheads in enumerate(groups):
                    for kk, hh in enumerate(gheads):
                        if hh < hA:
                            src_ap = ag_a_out[:, :, hh * dh : (hh + 1) * dh]
                        else:
                            src_ap = ag_b_out[
                                :, :, (hh - hA) * dh : (hh - hA + 1) * dh
                            ]
                        for half in range(2):
                            rs, js = half * HR, half * HR * 4
                            nc.sync.dma_start(
                                whrow_g[gi][:, js : js + 4 * HR, kk, 1 : dh + 1],
                                src_ap[rs : rs + HR].rearrange(
                                    "r (k p) d -> p (r k) d", p=P
                                ),
                            )

                # ---- (F) attention per head group
                xnext = xTown_pool.tile([fout, ROWS], dt.float32r, tag="xTown")
                for gi, gs in enumerate(groups):
                    ng = len(gs)
                    att_acc = []
                    for _k in gs:
                        att_t = attps.tile([SW, ROWS], dt.float32, tag="att")
                        att_acc.append(att_t)
                    for jt2 in range(0, JT, 2):
                        l_jt = work.tile([P, ng, 2 * ROWS], dt.bfloat16, tag="l_jt")
                        for k, hh in enumerate(gs):
                            nc.vector._custom_dve(
                                LEAKY2_BIAS_ADDMASK,
                                out=l_jt[:, k, :].rearrange(
                                    "p (s n) -> p s n", s=2
                                ),
                                in0=srep_all[:, hh, :]
                                .rearrange("p (o n) -> p o n", o=1)
                                .broadcast_to([P, 2, ROWS]),
                                in1=maskT[:, jt2 : jt2 + 2, :].rearrange(
                                    "p s n -> p (s n)"
                                ),
                                s0=d_sb[:, jt2, hh : hh + 1].bitcast(dt.float32),
                                s1=d_diff[:, jt2 // 2, hh : hh + 1],
                                imm2=ALPHA,
                            )
                        p_jt = work.tile([P, ng, 2 * ROWS], dt.float32r, tag="p_jt")
                        nc.scalar.activation(p_jt[:], l_jt[:], AF.Exp)
                        for s in range(2):
                            for k, hh in enumerate(gs):
                                nc.tensor.matmul(
                                    att_acc[k][:],
                                    whrow_g[gi][:, jt2 + s, k, :],
                                    p_jt[:, k, s * ROWS : (s + 1) * ROWS],
                                    start=(jt2 == 0 and s == 0),
                                    stop=(jt2 == JT - 2 and s == 1),
                                )
                    # epilogue per head: normalize; write into xnext via DMA
                    for k, hh in enumerate(gs):
                        o_sb = small.tile([SW, ROWS], dt.float32, tag="o_sb")
                        nc.scalar.copy(o_sb[:], att_acc[k][:])
                        r_sb = small.tile([1, ROWS], dt.float32, tag="vec1")
                        nc.vector.reciprocal_approx_fast(r_sb[:], o_sb[0:1, :])
                        r_sbr = small.tile([1, ROWS], dt.float32r, tag="vec1r")
                        nc.vector.tensor_copy(r_sbr[:], r_sb[:])
                        ps_rr = sps.tile([SW, ROWS], dt.float32, tag="ps_rep")
                        nc.tensor.matmul(
                            ps_rr[:], ones_row[0:1, 0:SW],
                            r_sbr[:],
                            start=True, stop=True,
                        )
                        onorm = small.tile([SW, ROWS], dt.float32r, tag="onorm")
                        nc.vector.tensor_mul(onorm[:], o_sb[:], ps_rr[:])
                        nc.sync.dma_start(
                            xnext[hh * dh : (hh + 1) * dh, :], onorm[1 : dh + 1, :]
                        )

                if elu:
                    # elu(x) = max(x,0) - 1 + exp(min(x,0)), batched full-width
                    mmin = small.tile([fout, ROWS], dt.float32, tag="emm")
                    nc.vector.tensor_scalar(
                        mmin[:], xnext[:], 0.0, None, ALU.min
                    )
                    emin = small.tile([fout, ROWS], dt.float32, tag="emm2")
                    nc.scalar.activation(emin[:], mmin[:], AF.Exp)
                    nc.vector.tensor_scalar(
                        mmin[:], xnext[:], 0.0, -1.0, ALU.max, ALU.add
                    )
                    nc.vector.tensor_add(xnext[:], mmin[:], emin[:])

                if is_last:
                    psum_final = small.tile([fout, 1], dt.float32, tag="vec1f")
                    nc.vector.reduce_sum(
                        psum_final[:], xnext[:], axis=mybir.AxisListType.X
                    )
                    nc.sync.dma_start(pool_out[:], psum_final[:])
                else:
                    ftiles = []
                    for ft in range((fout + P - 1) // P):
                        fr = min(P, fout - ft * P)
                        ftiles.append(xnext[ft * P : ft * P + fr, :])
                    xTown_cur = ftiles

    nc.finalize()
    return nc


_NC_CACHE = None
_last_in_maps = None


def postprocess(results, inputs):
    fc_w = np.asarray(inputs["fc_w"], dtype=np.float32)
    fc_b = np.asarray(inputs["fc_b"], dtype=np.float32)
    total = np.zeros((8,), dtype=np.float32)
    for c in range(NCORE):
        total += results[c]["pool_part"][:, 0]
    pooled = total / np.float32(N)
    return (pooled @ fc_w + fc_b).astype(np.float32)


def kernel(**inputs):
    global _NC_CACHE, _last_in_maps
    node_features = np.asarray(inputs["node_features"], dtype=np.float32)
    adj = np.asarray(inputs["adj_mat"], dtype=np.int32)
    adjT = np.ascontiguousarray(adj.T)
    x0T = np.ascontiguousarray(node_features.T)  # [256, N]

    wext = {}
    ws = {}
    for li, (fin, fout, h, concat, _elu) in enumerate(CFG, start=1):
        dh = fout // h if concat else fout
        hA, hB, CWa, CWb = _layer_split(h, dh)
        W = np.asarray(inputs[f"W{li}"], dtype=np.float32)  # [h, fin, dh]
        a_src = np.asarray(inputs[f"a_src{li}"], dtype=np.float32)  # [h, dh]
        a_dst = np.asarray(inputs[f"a_dst{li}"], dtype=np.float32)
        wcat = W.transpose(1, 0, 2).reshape(fin, h * dh)
        wd = np.einsum("hfd,hd->fh", W, a_dst).astype(np.float32)
        wsrc = np.einsum("hfd,hd->fh", W, a_src).astype(np.float32)
        wcat_full = np.concatenate([wcat[:, : hA * dh], wd, wcat[:, hA * dh :]], axis=1)
        if wcat_full.shape[1] % 2:
            wcat_full = np.concatenate(
                [wcat_full, np.zeros((fin, 1), np.float32)], axis=1
            )
        wext[li] = np.ascontiguousarray(wcat_full)
        ws[li] = np.ascontiguousarray(wsrc)

    in_maps = []
    for c in range(NCORE):
        m = {
            "adjT": np.ascontiguousarray(adjT[:, c * ROWS : (c + 1) * ROWS]).astype(np.int8),
            "x0T_own": np.ascontiguousarray(x0T[:, c * ROWS : (c + 1) * ROWS]),
        }
        for li in range(1, 6):
            m[f"wext{li}"] = wext[li]
            m[f"ws{li}"] = ws[li]
        in_maps.append(m)

    if _NC_CACHE is None:
        _NC_CACHE = build_kernel()
    nc = _NC_CACHE
    _last_in_maps = in_maps

    res = run_bass_kernel_spmd(nc, in_maps, list(range(NCORE)))
    return postprocess(res.results, inputs)


# revision 26
# speedup vs baseline: 1.0182x; 1.0062x over previous
"""GAT (5-layer, dense-adjacency) Trainium2 kernel, sharded across 8 NeuronCores.

Sharding: query-node rows split 512/core. Per layer each core projects its
own row-block (Wh + d), AllGathers the fp32 projections, and computes its
row-block of attention with a fused DVE op (leaky(s+d)+addmask), ACT exp,
and a single float32r matmul per (j-tile, head) whose stationary packs
[ones | Wh] so softmax denominators ride along for free.
"""

import numpy as np

import concourse.bacc as bacc
import concourse.mybir as mybir
import concourse.tile as tile
from concourse.bass_utils import run_bass_kernel_spmd

import concourse.dve_ops as dve_ops
from concourse.dve_spec import Spec, Src0, Src1, C0, C1, C2, maxx, lower, PageIdx
from concourse.dve_spec import _has_src1 as _spec_has_src1
from concourse.dve_uop import DveOpSpec

dt = mybir.dt
AF = mybir.ActivationFunctionType
ALU = mybir.AluOpType

# ---------------------------------------------------------------- constants
N = 4096
NCORE = 8
ROWS = N // NCORE  # 512 query rows per core
P = 128
JT = N // P  # 32 j-tiles
NEG = -30000.0  # additive mask for non-edges; exp(x-30000) == 0
ALPHA = 0.1
# (fin, fout, heads, concat, elu_after)
CFG = [
    (256, 128, 8, True, True),
    (128, 64, 8, True, True),
    (64, 32, 4, True, True),
    (32, 16, 1, True, False),
    (16, 8, 1, False, False),
]

# ---------------------------------------------------------------- custom op
LEAKY_BIAS_ADDMASK = dve_ops.DveOp(
    "LEAKY_BIAS_ADDMASK",
    Spec(
        body=maxx(Src0 + C0, (Src0 + C0) * C1) + Src1,
        reference=lambda in0, in1, s0, s1, imm2: (
            np.maximum(in0 + s0, (in0 + s0) * s1) + in1
        ).astype(np.float32),
    ),
    subdim=False,
    uops_sha={},
)


def _register_custom_op(op):
    if op.name in dve_ops._SUB_OPCODE_FOR_NAME:
        return
    idx = dve_ops._CUSTOM_DVE_ROW_BASE + len(dve_ops.OPS)
    assert idx < 0x20
    dve_ops.OPS.append(op)
    dve_ops.CUSTOM_DVE_SPECS[op.name] = op.spec
    dve_ops._SUB_OPCODE_FOR_NAME[op.name] = idx
    shas = {}
    for ver in ("v3", "v4"):
        try:
            s = DveOpSpec(
                name=op.name,
                opcode=idx,
                uops=lower(op.spec, ver=ver),
                rd1_en=_spec_has_src1(op.spec),
            )
            shas[ver] = s.sha(ver)
        except Exception:
            pass
    object.__setattr__(op, "uops_sha", shas)


_register_custom_op(LEAKY_BIAS_ADDMASK)


def _leaky2_ref(in0, in1, s0, s1, imm2):
    x = in0.reshape(in0.shape[0], 2, -1).astype(np.float32)
    d0 = np.asarray(s0).reshape(-1, 1)
    dd = np.asarray(s1).reshape(-1, 1)
    t = np.stack([x[:, 0, :] + d0, x[:, 1, :] + d0 + dd], axis=1)
    l = np.maximum(t, t * imm2) + in1.reshape(t.shape).astype(np.float32)
    return l.reshape(in0.shape).astype(np.float32)


_T2 = Src0 + PageIdx(C0, C1)
LEAKY2_BIAS_ADDMASK = dve_ops.DveOp(
    "LEAKY2_BIAS_ADDMASK",
    Spec(body=maxx(_T2, _T2 * C2) + Src1, reference=_leaky2_ref),
    subdim=True,
    uops_sha={},
)
_register_custom_op(LEAKY2_BIAS_ADDMASK)


def _layer_split(h, dh):
    """AG split: group A (first hA heads + all d cols), group B (rest)."""
    hA = min(1, h)
    hB = h - hA
    CWa = hA * dh + h
    CWb = hB * dh
    return hA, hB, CWa, CWb


def _att_groups(h, hA):
    """Attention head groups: first group = AG-A heads, rest from AG-B."""
    if h == 1:
        return [[0]]
    gs = [list(range(hA))]
    rest = list(range(hA, h))
    while rest:
        take = min(4, len(rest))
        gs.append(rest[:take])
        rest = rest[take:]
    return gs


# ---------------------------------------------------------------- builder
def build_kernel():
    nc = bacc.Bacc("TRN2", target_bir_lowering=False, debug=False)

    adjT = nc.dram_tensor("adjT", [N, ROWS], dt.int8, kind="ExternalInput")
    x0T_own = nc.dram_tensor("x0T_own", [256, ROWS], dt.float32r, kind="ExternalInput")
    wext_dram = {}
    ws_dram = {}
    for li, (fin, fout, h, concat, _elu) in enumerate(CFG, start=1):
        dh = fout // h if concat else fout
        CWp = h * dh + h + (h * dh + h) % 2
        wext_dram[li] = nc.dram_tensor(
            f"wext{li}", [fin, CWp], dt.float32r, kind="ExternalInput"
        )
        ws_dram[li] = nc.dram_tensor(f"ws{li}", [fin, h], dt.float32r, kind="ExternalInput")

    pool_out = nc.dram_tensor("pool_part", [8, 1], dt.float32, kind="ExternalOutput")

    with tile.TileContext(nc) as tc:
        with (
            tc.tile_pool(name="persist", bufs=1) as persist,
            tc.tile_pool(name="dram", bufs=1, space="DRAM") as drampool,
            tc.tile_pool(name="dramsh", bufs=1, space="DRAM") as drampool_sh,
            tc.tile_pool(name="xTown", bufs=3) as xTown_pool,
            tc.tile_pool(name="layerbuf", bufs=1) as layerbuf,
            tc.tile_pool(name="mstage", bufs=6) as mstage,
            tc.tile_pool(name="work", bufs=2) as work,
            tc.tile_pool(name="sflat", bufs=1) as sflat_pool,
            tc.tile_pool(name="small", bufs=2) as small,
            tc.tile_pool(name="whps", bufs=1, space="PSUM") as whps,
            tc.tile_pool(name="sps", bufs=1, space="PSUM") as sps,
            tc.tile_pool(name="attps", bufs=4, space="PSUM") as attps,
        ):
            # ---------------- persistent tiles
            maskT = persist.tile([P, JT, ROWS], dt.bfloat16, tag="maskT")
            ones_row32 = persist.tile([1, P], dt.float32, tag="ones_row32")
            nc.vector.memset(ones_row32[:], 1.0)
            ones_row = persist.tile([1, P], dt.float32r, tag="ones_row")
            nc.vector.tensor_copy(ones_row[:], ones_row32[:])
            ones_col = persist.tile([P, JT], dt.float32, tag="ones_col")
            nc.vector.memset(ones_col[:], 1.0)
            
            wext_sb = {}
            ws_sb = {}
            for li, (fin, fout, h, concat, _elu) in enumerate(CFG, start=1):
                dh = fout // h if concat else fout
                nft = (fin + P - 1) // P
                wext_sb[li] = []
                ws_sb[li] = []
                for ft in range(nft):
                    fr = min(P, fin - ft * P)
                    wt = persist.tile([fr, h * dh + h + (h * dh + h) % 2], dt.float32r, tag=f"wext{li}_{ft}")
                    nc.sync.dma_start(wt[:], wext_dram[li][ft * P : ft * P + fr, :])
                    wext_sb[li].append(wt)
                    st = persist.tile([fr, h], dt.float32r, tag=f"ws{li}_{ft}")
                    nc.sync.dma_start(st[:], ws_dram[li][ft * P : ft * P + fr, :])
                    ws_sb[li].append(st)

            # ---------------- L1 own activations from input
            xTown_cur = []
            for ft in range(2):
                to = xTown_pool.tile([P, ROWS], dt.float32r, tag="xTown")
                nc.sync.dma_start(to[:], x0T_own[ft * P : (ft + 1) * P, :])
                xTown_cur.append(to)

            for li, (fin, fout, h, concat, elu) in enumerate(CFG, start=1):
                dh = fout // h if concat else fout
                hdh = h * dh
                CW = hdh + h  # Wh values + d column(s)
                CWp = CW + CW % 2  # fp32r matmul needs even moving dim
                nft = (fin + P - 1) // P
                is_last = li == len(CFG)
                hA, hB, CWa, CWb = _layer_split(h, dh)
                SW = dh + 1  # stationary width per head: [ones | Wh]

                # ---- (A) own-block Wh (+d) for the 4 own j-chunks (fp32)
                own_sb = work.tile([P, 4, CWp], dt.float32r, tag="own_sb")
                for k in range(4):
                    pw = whps.tile([P, CWp], dt.float32, tag="pw")
                    for ft in range(nft):
                        fr = min(P, fin - ft * P)
                        nc.tensor.matmul(
                            pw[:],
                            xTown_cur[ft][0:fr, k * P : (k + 1) * P],
                            wext_sb[li][ft][:],
                            start=(ft == 0),
                            stop=(ft == nft - 1),
                        )
                    nc.scalar.copy(own_sb[:, k, :], pw[:])

                # ---- (C) AllGather fp32 projections (A then B)
                ag_a_in = drampool.tile([4 * P, CWa], dt.float32r, tag=f"again{li}")
                ag_a_out = drampool_sh.tile(
                    [NCORE, 4 * P, CWa], dt.float32r, tag=f"agaout{li}",
                    addr_space="Shared",
                )
                nc.sync.dma_start(
                    ag_a_in.rearrange("(k p) c -> p k c", p=P), own_sb[:, :, 0:CWa]
                )
                nc.gpsimd.collective_compute(
                    "AllGather",
                    mybir.AluOpType.bypass,
                    replica_groups=[list(range(NCORE))],
                    ins=[ag_a_in.opt()],
                    outs=[ag_a_out.opt()],
                )
                if hB:
                    ag_b_in = drampool.tile([4 * P, CWb], dt.float32r, tag=f"agbin{li}")
                    ag_b_out = drampool_sh.tile(
                        [NCORE, 4 * P, CWb], dt.float32r, tag=f"agbout{li}",
                        addr_space="Shared",
                    )
                    nc.sync.dma_start(
                        ag_b_in.rearrange("(k p) c -> p k c", p=P),
                        own_sb[:, :, CWa:CW],
                    )
                    nc.gpsimd.collective_compute(
                        "AllGather",
                        mybir.AluOpType.bypass,
                        replica_groups=[list(range(NCORE))],
                        ins=[ag_b_in.opt()],
                        outs=[ag_b_out.opt()],
                    )

                if li == 1:
                    # mask build overlaps L1's AllGather: adjT rows are j
                    # (host-transposed); cast+scale on gpsimd, DVE helps.
                    for jt in range(JT):
                        stage_i = mstage.tile([P, ROWS], dt.int8, tag="stage_i")
                        nc.sync.dma_start(stage_i[:], adjT[jt * P : (jt + 1) * P, :])
                        eng = nc.gpsimd if jt % 4 else nc.vector
                        eng.tensor_scalar(
                            maskT[:, jt, :], stage_i[:], -NEG, NEG, ALU.mult, ALU.add
                        )

                # ---- (B) s rows from own activations + srep broadcasts
                ps_s = sps.tile([h, ROWS], dt.float32, tag="ps_s")
                for ft in range(nft):
                    fr = min(P, fin - ft * P)
                    nc.tensor.matmul(
                        ps_s[:],
                        ws_sb[li][ft][:],
                        xTown_cur[ft][0:fr, :],
                        start=(ft == 0),
                        stop=(ft == nft - 1),
                    )
                s_rows = small.tile([h, ROWS], dt.float32r, tag="s_rows")
                nc.vector.tensor_copy(s_rows[:], ps_s[:])
                s_flat = sflat_pool.tile([1, h, ROWS], dt.float32r, tag="s_flat")
                nc.sync.dma_start(s_flat[:], s_rows[:])
                srep_all = layerbuf.tile([P, h, ROWS], dt.float32, tag="srep_all")
                for hh in range(h):
                    ps_rep = sps.tile([P, ROWS], dt.float32, tag="ps_rep")
                    nc.tensor.matmul(
                        ps_rep[:], ones_row[:],
                        s_flat[0:1, hh, :],
                        start=True, stop=True,
                    )
                    nc.scalar.copy(srep_all[:, hh, :], ps_rep[:])

                # ---- (D) unpack: direct strided DMAs into matmul layout
                # One whrow tile per attention group so group-A attention can
                # start while AG-B is still in flight (tile-granular deps).
                groups = _att_groups(h, hA)
                whrow_g = []
                for gi, gheads in enumerate(groups):
                    wg = layerbuf.tile(
                        [P, JT, len(gheads), SW], dt.float32r, tag=f"whrow{gi}"
                    )
                    for kk in range(len(gheads)):
                        nc.vector.tensor_copy(wg[:, :, kk, 0:1], ones_col[:, :])
                    whrow_g.append(wg)
                d_sb = layerbuf.tile([P, JT, h], dt.float32r, tag="d_sb")
                HR = NCORE // 2
                for half in range(2):
                    rs, js = half * HR, half * HR * 4
                    nc.sync.dma_start(
                        d_sb[:, js : js + 4 * HR, :],
                        ag_a_out[rs : rs + HR, :, hA * dh : hA * dh + h].rearrange(
                            "r (k p) h -> p (r k) h", p=P
                        ),
                    )
                d_diff = layerbuf.tile([P, JT // 2, h], dt.float32, tag="d_diff")
                nc.vector.tensor_sub(
                    d_diff[:],
                    d_sb[:, 1::2, :].bitcast(dt.float32),
                    d_sb[:, 0::2, :].bitcast(dt.float32),
                )
                for gi, gheads in enumerate(groups):
                    for kk, hh in enumerate(gheads):
                        if hh < hA:
                            src_ap = ag_a_out[:, :, hh * dh : (hh + 1) * dh]
                        else:
                            src_ap = ag_b_out[
                                :, :, (hh - hA) * dh : (hh - hA + 1) * dh
                            ]
                        for half in range(2):
                            rs, js = half * HR, half * HR * 4
                            nc.sync.dma_start(
                                whrow_g[gi][:, js : js + 4 * HR, kk, 1 : dh + 1],
                                src_ap[rs : rs + HR].rearrange(
                                    "r (k p) d -> p (r k) d", p=P
                                ),
                            )

                # ---- (F) attention per head group
                xnext = xTown_pool.tile([fout, ROWS], dt.float32r, tag="xTown")
                for gi, gs in enumerate(groups):
                    ng = len(gs)
                    att_acc = []
                    for _k in gs:
                        att_t = attps.tile([SW, ROWS], dt.float32, tag="att")
                        att_acc.append(att_t)
                    for jt2 in range(0, JT, 2):
                        l_jt = work.tile([P, ng, 2 * ROWS], dt.bfloat16, tag="l_jt")
                        for k, hh in enumerate(gs):
                            nc.vector._custom_dve(
                                LEAKY2_BIAS_ADDMASK,
                                out=l_jt[:, k, :].rearrange(
                                    "p (s n) -> p s n", s=2
                                ),
                                in0=srep_all[:, hh, :]
                                .rearrange("p (o n) -> p o n", o=1)
                                .broadcast_to([P, 2, ROWS]),
                                in1=maskT[:, jt2 : jt2 + 2, :].rearrange(
                                    "p s n -> p (s n)"
                                ),
                                s0=d_sb[:, jt2, hh : hh + 1].bitcast(dt.float32),
                                s1=d_diff[:, jt2 // 2, hh : hh + 1],
                                imm2=ALPHA,
                            )
                        p_jt = work.tile([P, ng, 2 * ROWS], dt.float32r, tag="p_jt")
                        nc.scalar.activation(p_jt[:], l_jt[:], AF.Exp)
                        for s in range(2):
                            for k, hh in enumerate(gs):
                                nc.tensor.matmul(
                                    att_acc[k][:],
                                    whrow_g[gi][:, jt2 + s, k, :],
                                    p_jt[:, k, s * ROWS : (s + 1) * ROWS],
                                    start=(jt2 == 0 and s == 0),
                                    stop=(jt2 == JT - 2 and s == 1),
                                )
                    # epilogue per head: normalize; write into xnext via DMA
                    for k, hh in enumerate(gs):
                        o_sb = small.tile([SW, ROWS], dt.float32, tag="o_sb")
                        nc.scalar.copy(o_sb[:], att_acc[k][:])
                        r_sb = small.tile([1, ROWS], dt.float32, tag="vec1")
                        nc.vector.reciprocal_approx_fast(r_sb[:], o_sb[0:1, :])
                        r_sbr = small.tile([1, ROWS], dt.float32r, tag="vec1r")
                        nc.vector.tensor_copy(r_sbr[:], r_sb[:])
                        ps_rr = sps.tile([SW, ROWS], dt.float32, tag="ps_rep")
                        nc.tensor.matmul(
                            ps_rr[:], ones_row[0:1, 0:SW],
                            r_sbr[:],
                            start=True, stop=True,
                        )
                        onorm = small.tile([SW, ROWS], dt.float32r, tag="onorm")
                        nc.vector.tensor_mul(onorm[:], o_sb[:], ps_rr[:])
                        nc.sync.dma_start(
                            xnext[hh * dh : (hh + 1) * dh, :], onorm[1 : dh + 1, :]
                        )

                if elu:
                    # elu(x) = max(x,0) - 1 + exp(min(x,0)), batched full-width
                    mmin = small.tile([fout, ROWS], dt.float32, tag="emm")
                    nc.vector.tensor_scalar(
                        mmin[:], xnext[:], 0.0, None, ALU.min
                    )
                    emin = small.tile([fout, ROWS], dt.float32, tag="emm2")
                    nc.scalar.activation(emin[:], mmin[:], AF.Exp)
                    nc.vector.tensor_scalar(
                        mmin[:], xnext[:], 0.0, -1.0, ALU.max, ALU.add
                    )
                    nc.vector.tensor_add(xnext[:], mmin[:], emin[:])

                if is_last:
                    psum_final = small.tile([fout, 1], dt.float32, tag="vec1f")
                    nc.vector.reduce_sum(
                        psum_final[:], xnext[:], axis=mybir.AxisListType.X
                    )
                    nc.sync.dma_start(pool_out[:], psum_final[:])
                else:
                    ftiles = []
                    for ft in range((fout + P - 1) // P):
                        fr = min(P, fout - ft * P)
                        ftiles.append(xnext[ft * P : ft * P + fr, :])
                    xTown_cur = ftiles

    nc.finalize()
    return nc


_NC_CACHE = None
_last_in_maps = None


def postprocess(results, inputs):
    fc_w = np.asarray(inputs["fc_w"], dtype=np.float32)
    fc_b = np.asarray(inputs["fc_b"], dtype=np.float32)
    total = np.zeros((8,), dtype=np.float32)
    for c in range(NCORE):
        total += results[c]["pool_part"][:, 0]
    pooled = total / np.float32(N)
    return (pooled @ fc_w + fc_b).astype(np.float32)


def kernel(**inputs):
    global _NC_CACHE, _last_in_maps
    node_features = np.asarray(inputs["node_features"], dtype=np.float32)
    adj = np.asarray(inputs["adj_mat"], dtype=np.int32)
    adjT = np.ascontiguousarray(adj.T)
    x0T = np.ascontiguousarray(node_features.T)  # [256, N]

    wext = {}
    ws = {}
    for li, (fin, fout, h, concat, _elu) in enumerate(CFG, start=1):
        dh = fout // h if concat else fout
        hA, hB, CWa, CWb = _layer_split(h, dh)
        W = np.asarray(inputs[f"W{li}"], dtype=np.float32)  # [h, fin, dh]
        a_src = np.asarray(inputs[f"a_src{li}"], dtype=np.float32)  # [h, dh]
        a_dst = np.asarray(inputs[f"a_dst{li}"], dtype=np.float32)
        wcat = W.transpose(1, 0, 2).reshape(fin, h * dh)
        wd = np.einsum("hfd,hd->fh", W, a_dst).astype(np.float32)
        wsrc = np.einsum("hfd,hd->fh", W, a_src).astype(np.float32)
        wcat_full = np.concatenate([wcat[:, : hA * dh], wd, wcat[:, hA * dh :]], axis=1)
        if wcat_full.shape[1] % 2:
            wcat_full = np.concatenate(
                [wcat_full, np.zeros((fin, 1), np.float32)], axis=1
            )
        wext[li] = np.ascontiguousarray(wcat_full)
        ws[li] = np.ascontiguousarray(wsrc)

    in_maps = []
    for c in range(NCORE):
        m = {
            "adjT": np.ascontiguousarray(adjT[:, c * ROWS : (c + 1) * ROWS]).astype(np.int8),
            "x0T_own": np.ascontiguousarray(x0T[:, c * ROWS : (c + 1) * ROWS]),
        }
        for li in range(1, 6):
            m[f"wext{li}"] = wext[li]
            m[f"ws{li}"] = ws[li]
        in_maps.append(m)

    if _NC_CACHE is None:
        _NC_CACHE = build_kernel()
    nc = _NC_CACHE
    _last_in_maps = in_maps

    res = run_bass_kernel_spmd(nc, in_maps, list(range(NCORE)))
    return postprocess(res.results, inputs)
